# revision 6
# baseline (speedup 1.0000x reference)
# NetVLAD pooling kernel for Trainium2 (Bass/Tile), 8-core data-parallel over B.
#
# reference:
#   logits = x @ assign_w + assign_b          # (B, T, K)
#   a = softmax(logits, axis=-1)
#   vlad[b,k,d] = sum_t a[b,t,k] * x[b,t,d] - (sum_t a[b,t,k]) * centroids[k,d]
#   out = l2_normalize(vlad, axis=-1).reshape(B, K*D)
#
# Per-core layout (4 batches each), software-pipelined over 512-token blocks:
#   stage A (block n):   DMA x block natural [t,d] f32; PE transposes the
#                        *truncated-bf16 view* of x (strided hi-short APs, so
#                        bf16 transpose rate, no conversion pass) -> xT bf16
#                        in PSUM; DVE copies xT -> SBUF (bf16 2x mode)
#   stage B (block n-1): GEMM1 logitsT[k=64,t=512] = w_bf.T @ xT (bf16);
#                        ACT exp(+bias) -> e bf16; PE [eT|s] = e.T @ [I|1|0]
#                        (bf16 moving, 66 cols); DVE reciprocal of s + batched
#                        raw eT copy -> SBUF; Pool (gpsimd) scales a = eT * 1/s
#                        into persistent zero-padded f32r slots
#   stage C (block n-2): GEMM2 vlad[k,258] += a[t,k].T @ [x|1|1][t,258] (f32r
#                        full-precision x) accumulated per batch
#   epilogue (deferred to the end, so ACT only ever loads the Exp table during
#   the main loop): vlad -= a_sum*c; L2-normalize over d; DMA out.
#
# softmax max-subtraction is skipped: logits ~ N(0, 0.8^2) so exp() is safe in
# f32, and softmax is shift-invariant (matches the reference up to rounding).
# x is truncated (not rounded) to bf16 only on the GEMM1/logits path; GEMM2
# consumes x at full f32r precision.

import numpy as np

import concourse.bass as bass
import concourse.tile as tile
from concourse import mybir
from concourse.bass_utils import run_bass_kernel_spmd
from concourse.masks import make_identity

B, T, D, K = 32, 4096, 256, 64
NCORES = 8
BPC = B // NCORES          # batches per core
TBLK = 512                 # tokens per pipeline block
NBLK = T // TBLK
NSUB = TBLK // 128         # 128-token subtiles per block
NXSLOT = 4                 # persistent x slots (ones columns written once)
NASLOT = 3                 # persistent a slots (zero pad written once)
F32 = mybir.dt.float32
F32R = mybir.dt.float32r
BF16 = mybir.dt.bfloat16

_FNS = mybir.ActivationFunctionType


def _split_multi_waits(nc, max_waits=1):
    """The walrus build in this container rejects instructions carrying more
    than one sync wait ("Too many sync wait commands" in setupSyncWait).
    Tile's kernel-tail drain aggregates one wait per live semaphore, so split
    any multi-wait instruction into a chain of single-wait NOPs in front of it.
    """
    for f in nc.m.functions:
        for blk in f.blocks:
            insts = blk.instructions
            if not any(
                i.sync_info and i.sync_info.on_wait and len(i.sync_info.on_wait) > max_waits
                for i in insts
            ):
                continue
            new = []
            for inst in insts:
                si = inst.sync_info
                if si is not None and si.on_wait and len(si.on_wait) > max_waits:
                    waits = list(si.on_wait)
                    for k, w in enumerate(waits[:-max_waits]):
                        nop = mybir.InstNoOp(name=f"{inst.name}-wsplit{k}", ins=[], outs=[])
                        nop.engine = inst.engine
                        nop.sync_info = mybir.SyncInfo(on_wait=[w], on_update=[])
                        new.append(nop)
                    inst.sync_info = mybir.SyncInfo(
                        on_wait=waits[-max_waits:], on_update=list(si.on_update)
                    )
                new.append(inst)
            blk.instructions = new


def _hi_bf16(ap):
    """Truncated-bf16 view of an f32 AP: the high short of each f32 element
    (little-endian), as a stride-2 bf16 access pattern."""
    return ap.bitcast(BF16).rearrange("p (d two) -> p d two", two=2)[:, :, 1]


def build(reps=1, use_f32r="gv", hw_loop=False):
    del use_f32r, hw_loop  # single config; kept for test.py compatibility

    nc = bass.Bass()
    x_h = nc.declare_dram_parameter("x", [BPC, T, D], F32, isOutput=False)
    w_h = nc.declare_dram_parameter("assign_w", [D, K], F32, isOutput=False)
    b_h = nc.declare_dram_parameter("assign_b", [K, 1], F32, isOutput=False)
    c_h = nc.declare_dram_parameter("centroids", [K, D], F32, isOutput=False)
    o_h = nc.declare_dram_parameter("out", [BPC, K * D], F32, isOutput=True)

    x_ap, w_ap, b_ap, c_ap, o_ap = (h.ap() for h in (x_h, w_h, b_h, c_h, o_h))

    with tile.TileContext(nc) as tc:
        with (
            tc.tile_pool(name="consts", bufs=1) as consts,
            tc.tile_pool(name="xts", bufs=2) as xts,
            tc.tile_pool(name="esb", bufs=2) as esb,
            tc.tile_pool(name="asb", bufs=2) as asb,
            tc.tile_pool(name="epi", bufs=2) as epi,
            tc.tile_pool(name="ps_t", bufs=2, space="PSUM") as ps_t,
            tc.tile_pool(name="ps_l", bufs=2, space="PSUM") as ps_l,
            tc.tile_pool(name="ps_a", bufs=2, space="PSUM") as ps_a,
            tc.tile_pool(name="ps_v", bufs=2, space="PSUM") as ps_v,
        ):
            ident_bf = consts.tile([128, 128], BF16, tag="ident_bf")
            make_identity(nc, ident_bf)

            # [I_64 | 1 | 0] bf16: transposes e back to [t,k] + row sums
            eyeones = consts.tile([K, K + 2], BF16, tag="eyeones")
            make_identity(nc, eyeones[:, 0:K])
            nc.gpsimd.memset(eyeones[:, K : K + 1], 1.0)
            nc.gpsimd.memset(eyeones[:, K + 1 : K + 2], 0.0)

            # w as bf16 [128, 2, 64] (stationary; bf16 needs no column pad)
            w_f32 = consts.tile([128, 2, K], F32, tag="w_f32")
            nc.sync.dma_start(
                out=w_f32, in_=w_ap.rearrange("(c p) k -> p c k", p=128)
            )
            w_bf = consts.tile([128, 2, K], BF16, tag="w_bf")
            nc.vector.tensor_copy(out=w_bf, in_=w_f32)

            b_sb = consts.tile([K, 1], F32, tag="b_sb")
            nc.sync.dma_start(out=b_sb, in_=b_ap)
            c_sb = consts.tile([K, D], F32, tag="c_sb")
            nc.sync.dma_start(out=c_sb, in_=c_ap)

            # persistent x slots: ones columns (GEMM2 a_sum trick) written once
            x_slot = [
                consts.tile([128, NSUB, D + 2], F32, tag=f"x_slot{i}", name=f"x_slot{i}")
                for i in range(NXSLOT)
            ]
            ones_st = consts.tile([128, NSUB, 2], F32, tag="ones_st")
            nc.gpsimd.memset(ones_st, 1.0)
            for xs in x_slot:
                nc.vector.tensor_copy(
                    out=xs.bitcast(F32R)[:, :, D : D + 2], in_=ones_st
                )

            # persistent a slots: f32r stationary needs 128 cols; zero the
            # [K:128] pad once
            a_slot = [
                consts.tile([128, NSUB, 128], F32, tag=f"a_slot{i}", name=f"a_slot{i}")
                for i in range(NASLOT)
            ]
            zpad_st = consts.tile([128, NSUB, 128 - K], F32, tag="zpad_st")
            nc.gpsimd.memset(zpad_st, 0.0)
            for asl in a_slot:
                nc.vector.tensor_copy(
                    out=asl.bitcast(F32R)[:, :, K:128], in_=zpad_st
                )

            v_raw = [
                consts.tile([K, D + 2], F32, tag=f"v_raw{i}", name=f"v_raw{i}") for i in range(BPC)
            ]

            for _rep in range(reps):
                NTOT = BPC * NBLK
                ps_v_cur = [None]

                def stage_a(g):
                    s = g % NXSLOT
                    b_i, blk = divmod(g, NBLK)
                    xs = x_slot[s]
                    nc.sync.dma_start(
                        out=xs.bitcast(F32R)[:, :, 0:D],
                        in_=x_ap[b_i, blk * TBLK : (blk + 1) * TBLK, :]
                        .rearrange("(n p) d -> p n d", p=128)
                        .bitcast(F32R),
                    )
                    xT_ps = ps_t.tile([128, 2, TBLK], BF16)
                    for jd in range(2):
                        for jt in range(NSUB):
                            nc.tensor.transpose(
                                out=xT_ps[:, jd, jt * 128 : (jt + 1) * 128],
                                in_=_hi_bf16(
                                    xs[:, jt, jd * 128 : (jd + 1) * 128]
                                ),
                                identity=ident_bf,
                            )
                    xT_sb = xts.tile([128, 2, TBLK], BF16)
                    # both copies on DVE: bf16 packed 2x mode makes them cheap
                    nc.vector.tensor_copy(out=xT_sb[:, 0, :], in_=xT_ps[:, 0, :])
                    nc.vector.tensor_copy(out=xT_sb[:, 1, :], in_=xT_ps[:, 1, :])
                    return xT_sb

                def stage_b(g, xT_sb):
                    sa = g % NASLOT
                    l_ps = ps_l.tile([K, TBLK], F32)
                    for jd in range(2):
                        nc.tensor.matmul(
                            out=l_ps,
                            lhsT=w_bf[:, jd, :],
                            rhs=xT_sb[:, jd, :],
                            start=(jd == 0),
                            stop=(jd == 1),
                            skip_group_check=True,
                        )
                    e_sb = esb.tile([K, TBLK], BF16)
                    nc.scalar.activation(
                        out=e_sb, in_=l_ps, func=_FNS.Exp, bias=b_sb, scale=1.0
                    )
                    a_ps = ps_a.tile([128, NSUB, K + 2], F32)
                    for jt in range(NSUB):
                        nc.tensor.matmul(
                            out=a_ps[:, jt, :],
                            lhsT=e_sb[:, jt * 128 : (jt + 1) * 128],
                            rhs=eyeones,
                            start=True,
                            stop=True,
                            skip_group_check=True,
                        )
                    rs = asb.tile([128, NSUB, 1], F32, tag="rs")
                    nc.vector.reciprocal(out=rs, in_=a_ps[:, :, K : K + 1])
                    araw = asb.tile([128, NSUB, K], F32, tag="araw")
                    # raw eT batch copy on ACT (exp's engine, has headroom)
                    nc.scalar.copy(out=araw, in_=a_ps[:, :, 0:K])
                    asl = a_slot[sa]
                    for jt in range(NSUB):
                        nc.gpsimd.tensor_scalar_mul(
                            out=asl.bitcast(F32R)[:, jt, 0:K],
                            in0=araw[:, jt, :],
                            scalar1=rs[:, jt, :],
                        )
                    return sa

                def stage_c(g, sa):
                    b_i, blk = divmod(g, NBLK)
                    if blk == 0:
                        ps_v_cur[0] = ps_v.tile([128, D + 2], F32, name="v_ps")
                    v_ps = ps_v_cur[0]
                    s = g % NXSLOT
                    asl = a_slot[sa]
                    xs = x_slot[s]
                    for jt in range(NSUB):
                        nc.tensor.matmul(
                            out=v_ps,
                            lhsT=asl.bitcast(F32R)[:, jt, :],
                            rhs=xs.bitcast(F32R)[:, jt, :],
                            start=(blk == 0 and jt == 0),
                            stop=(blk == NBLK - 1 and jt == NSUB - 1),
                            skip_group_check=True,
                        )
                    if blk == NBLK - 1:
                        nc.vector.tensor_copy(
                            out=v_raw[b_i], in_=v_ps[0:K, :]
                        )

                pend_b = None  # (g, xT_sb)
                pend_c = None  # (g, sa)
                for g in range(NTOT + 2):
                    if g < NTOT:
                        xT_sb = stage_a(g)
                    if pend_b is not None:
                        gb, xT = pend_b
                        sa = stage_b(gb, xT)
                        pend_b_out = (gb, sa)
                    else:
                        pend_b_out = None
                    if pend_c is not None:
                        stage_c(*pend_c)
                    pend_c = pend_b_out
                    pend_b = (g, xT_sb) if g < NTOT else None

                # deferred epilogue: vlad = v - a_sum*c, L2-normalize over d.
                # ACT runs only Square and Sqrt here (one table load each).
                ssq4 = epi.tile([K, BPC], F32, tag="ssq4")
                v_sb = []
                for b_i in range(BPC):
                    tmp = epi.tile([K, D], F32, tag=f"tmp{b_i}")
                    nc.vector.tensor_scalar(
                        out=tmp,
                        in0=c_sb,
                        scalar1=v_raw[b_i][:, D : D + 1],
                        scalar2=None,
                        op0=mybir.AluOpType.mult,
                    )
                    v = epi.tile([K, D], F32, tag=f"v{b_i}")
                    nc.vector.tensor_sub(out=v, in0=v_raw[b_i][:, 0:D], in1=tmp)
                    v_sb.append(v)
                    sq = epi.tile([K, D], F32, tag=f"sq{b_i}")
                    nc.scalar.activation(
                        out=sq, in_=v, func=_FNS.Square,
                        accum_out=ssq4[:, b_i : b_i + 1],
                    )
                nrm4 = epi.tile([K, BPC], F32, tag="nrm4")
                nc.scalar.activation(out=nrm4, in_=ssq4, func=_FNS.Sqrt)
                nc.vector.tensor_scalar_max(out=nrm4, in0=nrm4, scalar1=1e-12)
                rn4 = epi.tile([K, BPC], F32, tag="rn4")
                nc.vector.reciprocal(out=rn4, in_=nrm4)
                for b_i in range(BPC):
                    o_sb = epi.tile([K, D], F32, tag=f"o{b_i}")
                    nc.vector.tensor_scalar_mul(
                        out=o_sb, in0=v_sb[b_i], scalar1=rn4[:, b_i : b_i + 1]
                    )
                    nc.sync.dma_start(
                        out=o_ap[b_i].rearrange("(k d) -> k d", d=D), in_=o_sb
                    )

    _split_multi_waits(nc)
    return nc


_nc_cache = {}


def _get_nc(reps=1, use_f32r="gv"):
    key = reps
    if key not in _nc_cache:
        _nc_cache[key] = build(reps=reps)
    return _nc_cache[key]


def _in_maps(x, centroids, assign_w, assign_b):
    x = np.ascontiguousarray(x, dtype=np.float32)
    w = np.ascontiguousarray(assign_w, dtype=np.float32)
    b = np.ascontiguousarray(assign_b, dtype=np.float32).reshape(K, 1)
    c = np.ascontiguousarray(centroids, dtype=np.float32)
    return [
        {
            "x": x[i * BPC : (i + 1) * BPC],
            "assign_w": w,
            "assign_b": b,
            "centroids": c,
        }
        for i in range(NCORES)
    ]


def kernel(x, centroids, assign_w, assign_b):
    nc = _get_nc(1)
    res = run_bass_kernel_spmd(
        nc, _in_maps(x, centroids, assign_w, assign_b), core_ids=list(range(NCORES))
    )
    return np.concatenate([res.results[i]["out"] for i in range(NCORES)], axis=0)
